# revision 34
# baseline (speedup 1.0000x reference)
"""Block-diagonal GRU cell for Trainium2, data-parallel over 8 NeuronCores.

Math (per batch row b, block j of 8, block size 256):
    wx  = x @ W_ir.T + b_ir_lin + b_ir          # [B, 6144], gates r|z|n global-chunked
    wh  = hb_j @ W_h[j].T + b_hr_j              # per block, local r|z|n chunks of 256
    r   = sigmoid(wxr + whr)
    z   = sigmoid(wxz + whz)
    n   = tanh(wxn + r * whn)
    h'  = (1-z)*hb + z*n

Device strategy (per core, batch-sharded 1024 rows):
  - Mixed fp8/fp16 matmuls. The r/z gates (x- and h-projections) and the
    n-gate h-projection run in fp8e4m3 with MatmulPerfMode.DoubleRow
    (2 contraction rows/cycle = 2x bf16 rate); the n-gate x-projection
    stays fp16 (it dominates output error: all-fp8 sims at rel 2.1e-2 vs
    the 2e-2 budget, this mix sims at 1.24e-2). PE work: 311296 cycles
    ~= 130us vs 205us all-fp16.
  - fp8 weights are pre-scaled by S=64 on host (W~N(0,0.02) sits at the
    e4m3 subnormal boundary; x64 moves it to N(0,1.28)). The 1/S folds
    into the activation `scale` operand. The fp16 n-gate weights get the
    same x64 so PSUM bank B is uniformly scaled.
  - PSUM bank A [128,512] accumulates 64*(wxr+whr | wxz+whz) in one
    accumulation group (4 DoubleRow x-chunks + 1 DoubleRow h-chunk).
    Bank B holds 64*[wxn | whn] (whn separate: r multiplies only whn).
  - Epilogue is fp16 (2-byte dtypes unlock DVE 2x perf modes) and uses
    the real Tanh table (same ACT table set as Sigmoid on trn2):
        rz = sigmoid(A/S)            ACT
        t3 = r * B.whn               DVE   (= S*(r*whn))
        t4 = B.wxn + t3              DVE
        n  = tanh(t4/S)              ACT
        d  = n - hb                  DVE
        t5 = z * d                   DVE
        oj = t5 + hb                 GPSIMD
    Output is stored fp16 and upcast on host (rounding ~2e-4, noise
    floor here is ~1.2e-2 from fp8).
  - Host layouts make every DMA contiguous >=1KB per partition line:
    xt8/xt16 [P, MT, K1, 128] (per-m-tile slices), ht8 [P, MT, JP, 4, 128]
    (per (m, block-pair) slices), weights [P, NB, K1, cols] (per-block
    slices), whrz/whn loaded whole.
  - Loop nest is j-outer / m-inner with j-column-major weight DMAs and
    per-(m,block-pair) stream tiles; next-block weights prefetch at the
    top of the previous block's m-loop, next-pair streams right after
    the slots they need release.
  - The epilogue is software-pipelined one (j,m) item behind the
    matmuls: the in-order ACT queue otherwise serializes on the
    rz -> DVE t3/t4 -> tanh roundtrip (~2us/item, the same as the PE's
    rate) and drains a ~10us tail after the last matmul.
  - Loads ride the SP HWDGE ring; stores are batched per (m, pair)
    [128,512] fp16 on the ACT HWDGE ring. Measured pitfall: gpsimd
    dma_start is software-DGE (~1us of Pool-sequencer descriptor
    generation per store) — keep stores on a HWDGE ring (SP/ACT).

Measured on 8 trn2 cores: 166344 ns best, rel err 1.2446e-2
(sim-predicted 1.2446e-2; fixed seed makes this deterministic). The
same NEFF lands bimodally at ~166us or ~193us depending on chip clock
state at launch (all engines uniformly ~18% slower in the slow state;
not caused by the program). bf16 baseline was 241842 ns.
"""

import sys

if "/opt/trn_rl_repo" not in sys.path:
    sys.path.insert(0, "/opt/trn_rl_repo")

import numpy as np

B, IN, H, NB = 8192, 1024, 2048, 8
BS = H // NB  # 256
NCORES = 8
BC = B // NCORES  # 1024 rows per core
P = 128
S = 64.0  # fp8 weight prescale
PIPELINE_EPILOGUE = True

_BUILD_CACHE = {}


def build_nc(bc=BC, has_bias=False):
    """Build the Bass program for one core (SPMD: same program on all 8)."""
    key = (bc, has_bias)
    if key in _BUILD_CACHE:
        return _BUILD_CACHE[key]

    from contextlib import ExitStack

    import concourse.bacc as bacc
    import concourse.mybir as mybir
    import concourse.tile as tile

    f8 = mybir.dt.float8e4
    f16 = mybir.dt.float16
    f32 = mybir.dt.float32
    SIG = mybir.ActivationFunctionType.Sigmoid
    TANH = mybir.ActivationFunctionType.Tanh
    DR = mybir.MatmulPerfMode.DoubleRow

    K1 = IN // P  # 8 contraction chunks for the x projection
    K2 = BS // P  # 2 contraction chunks per block for the h projection
    MT = bc // P  # m-tiles (128 batch rows each)
    NJP = NB // 2  # block pairs

    nc = bacc.Bacc(target_bir_lowering=False)

    xt8_d = nc.dram_tensor("xt8", [P, MT, K1, P], f8, kind="ExternalInput").ap()
    xt16_d = nc.dram_tensor("xt16", [P, MT, K1, P], f16, kind="ExternalInput").ap()
    ht8_d = nc.dram_tensor("ht8", [P, MT, NJP, 2 * K2, P], f8, kind="ExternalInput").ap()
    h16_d = nc.dram_tensor("h16", [bc, H], f16, kind="ExternalInput").ap()
    wrz8_d = nc.dram_tensor("wrz8", [P, NB, K1, 2 * BS], f8, kind="ExternalInput").ap()
    wn16_d = nc.dram_tensor("wn16", [P, NB, K1, BS], f16, kind="ExternalInput").ap()
    whrz8_d = nc.dram_tensor("whrz8", [P, K2, NB, 2 * BS], f8, kind="ExternalInput").ap()
    whn8_d = nc.dram_tensor("whn8", [P, K2, NB, BS], f8, kind="ExternalInput").ap()
    if has_bias:
        brz_d = nc.dram_tensor("brz", [1, NB * 2 * BS], f32, kind="ExternalInput").ap()
        bxn_d = nc.dram_tensor("bxn", [1, NB * BS], f32, kind="ExternalInput").ap()
        bhn_d = nc.dram_tensor("bhn", [1, NB * BS], f32, kind="ExternalInput").ap()
    out = nc.dram_tensor("out", [bc, H], f16, kind="ExternalOutput").ap()

    with tile.TileContext(nc) as tc, ExitStack() as ctx:
        wpool = ctx.enter_context(tc.tile_pool(name="wres", bufs=1))
        spool = ctx.enter_context(tc.tile_pool(name="stream", bufs=3))
        psA = ctx.enter_context(tc.tile_pool(name="psA", bufs=4, space="PSUM"))
        psB = ctx.enter_context(tc.tile_pool(name="psB", bufs=4, space="PSUM"))
        epool = ctx.enter_context(tc.tile_pool(name="epi", bufs=4))
        opool = ctx.enter_context(tc.tile_pool(name="ostage", bufs=MT + 2))

        # ---- resident tiles ----
        xt8_sb = wpool.tile([P, MT, K1, P], f8, tag="xt8_sb")
        xt16_sb = wpool.tile([P, MT, K1, P], f16, tag="xt16_sb")
        wrz8_sb = wpool.tile([P, NB, K1, 2 * BS], f8, tag="wrz8_sb")
        wn16_sb = wpool.tile([P, NB, K1, BS], f16, tag="wn16_sb")
        whrz8_sb = wpool.tile([P, K2, NB, 2 * BS], f8, tag="whrz8_sb")
        whn8_sb = wpool.tile([P, K2, NB, BS], f8, tag="whn8_sb")

        def load_w_cols(j):
            nc.sync.dma_start(wrz8_sb[:, j], wrz8_d[:, j])
            nc.sync.dma_start(wn16_sb[:, j], wn16_d[:, j])
            nc.sync.dma_start(whrz8_sb[:, :, j], whrz8_d[:, :, j])
            nc.sync.dma_start(whn8_sb[:, :, j], whn8_d[:, :, j])

        # h16 rearranged so one DMA covers every m-tile of a block pair:
        # row m*128+p, col c -> [p, m, c]
        h16_r = h16_d.rearrange("(m p) c -> p m c", p=P)

        def load_jp_streams(jp, h_eng=None):
            # 2 triggers per block pair instead of 16: SP-queue trigger
            # serialization (~600ns each) was starving the PE early on
            ht_jp = spool.tile([P, MT, 2 * K2, P], f8, tag="ht_jp")
            nc.sync.dma_start(ht_jp[:], ht8_d[:, :, jp])
            h_jp = spool.tile([P, MT, 2 * BS], f16, tag="h_jp")
            psl = slice(2 * jp * BS, (2 * jp + 2) * BS)
            (h_eng or nc.sync).dma_start(h_jp[:], h16_r[:, :, psl])
            return ht_jp, h_jp

        # prewarm the ACT table set (sigmoid_and_others contains Tanh too)
        ws = wpool.tile([P, 1], f32, tag="ws")
        nc.vector.memset(ws[:], 0.0)
        nc.scalar.activation(ws[:], ws[:], SIG)

        # head, ordered by the in-order PE queue's need times. SP ring
        # carries the j0-critical pieces; the three bulk early loads ride
        # the (otherwise idle until ~11us) ACT ring so their transfers
        # overlap the SP ring's instead of queueing behind them.
        nc.sync.dma_start(xt8_sb[:, 0], xt8_d[:, 0])
        nc.sync.dma_start(wrz8_sb[:, 0, 0:2], wrz8_d[:, 0, 0:2])
        nc.scalar.dma_start(xt8_sb[:, 1:], xt8_d[:, 1:])
        jpstreams = {0: load_jp_streams(0, h_eng=nc.scalar)}
        nc.sync.dma_start(wrz8_sb[:, 0, 2:4], wrz8_d[:, 0, 2:4])
        nc.sync.dma_start(wrz8_sb[:, 0, 4:], wrz8_d[:, 0, 4:])
        nc.sync.dma_start(xt16_sb[:, 0], xt16_d[:, 0])
        nc.scalar.dma_start(xt16_sb[:, 1:], xt16_d[:, 1:])
        nc.sync.dma_start(wn16_sb[:, 0], wn16_d[:, 0])
        nc.sync.dma_start(whn8_sb[:, :, 0], whn8_d[:, :, 0])
        nc.sync.dma_start(whrz8_sb[:, :, 0], whrz8_d[:, :, 0])
        if has_bias:
            ones_sb = wpool.tile([1, P], f32, tag="ones_sb")
            nc.vector.memset(ones_sb[:], 1.0)
            brz_sb = wpool.tile([1, NB * 2 * BS], f32, tag="brz_sb")
            bxn_sb = wpool.tile([1, NB * BS], f32, tag="bxn_sb")
            bhn_sb = wpool.tile([1, NB * BS], f32, tag="bhn_sb")
            nc.sync.dma_start(brz_sb[:], brz_d[:])
            nc.sync.dma_start(bxn_sb[:], bxn_d[:])
            nc.sync.dma_start(bhn_sb[:], bhn_d[:])

        ostage = {}
        pending = None

        def finish_epilogue(j, m, rz, Bt, h_jp, oj2):
            jp, half_i = divmod(j, 2)
            half = slice(half_i * BS, (half_i + 1) * BS)
            msl = slice(m * P, (m + 1) * P)
            t3 = epool.tile([P, BS], f16, tag="t3")
            nc.vector.tensor_mul(t3[:], rz[:, 0:BS], Bt[:, BS : 2 * BS])
            t4 = epool.tile([P, BS], f16, tag="t4")
            nc.vector.tensor_add(t4[:], Bt[:, 0:BS], t3[:])
            tn = epool.tile([P, BS], f16, tag="tn")
            nc.scalar.activation(tn[:], t4[:], TANH, scale=1.0 / S)
            d = epool.tile([P, BS], f16, tag="d")
            nc.vector.tensor_sub(d[:], tn[:], h_jp[:, m, half])
            t5 = epool.tile([P, BS], f16, tag="t5")
            nc.vector.tensor_mul(t5[:], rz[:, BS : 2 * BS], d[:])
            nc.gpsimd.tensor_add(oj2[:, half], t5[:], h_jp[:, m, half])
            if half_i == 1:
                # one batched [128, 512] fp16 store per (m, block-pair) on
                # the ACT HWDGE ring. (gpsimd DMAs are software-DGE: ~1us
                # of Pool-sequencer descriptor generation each -- avoid.)
                psl = slice(2 * jp * BS, (2 * jp + 2) * BS)
                nc.scalar.dma_start(out[msl, psl], oj2[:, :])
                del ostage[m]

        for j in range(NB):
            jp, half_i = divmod(j, 2)
            jrz = slice(j * 2 * BS, (j + 1) * 2 * BS)
            jn = slice(j * BS, (j + 1) * BS)
            half = slice(half_i * BS, (half_i + 1) * BS)
            # prefetch the next block's weights ahead of the next pair's
            # stream loads on the SP queue
            if j + 1 < NB:
                load_w_cols(j + 1)
            if half_i == 0 and jp + 1 < NJP:
                # prefetch the next pair's streams (2 triggers); spool
                # bufs=3 holds released/current/next
                jpstreams[jp + 1] = load_jp_streams(jp + 1)
            for m in range(MT):
                msl = slice(m * P, (m + 1) * P)
                if half_i == 0:
                    ostage[m] = opool.tile(
                        [P, 2 * BS], f16, tag="oj2", name=f"oj2_{m}"
                    )
                ht_jp, h_jp = jpstreams[jp]
                A = psA.tile([P, 2 * BS], f32, tag="A")
                Bt = psB.tile([P, 2 * BS], f32, tag="B")
                # rz x-projection: fp8 DoubleRow over k-chunk pairs; B's
                # group start marks the whole bank pending-zero so the
                # h-side MMs overwrite-then-accumulate correctly.
                for k in range(K1 // 2):
                    nc.tensor.matmul(
                        A[:, :],
                        lhsT=xt8_sb[:, m, 2 * k : 2 * k + 2, :],
                        rhs=wrz8_sb[:, j, 2 * k : 2 * k + 2, :],
                        start=(k == 0), stop=False, perf_mode=DR,
                    )
                # n x-projection: fp16 (error-critical path)
                for k in range(K1):
                    nc.tensor.matmul(
                        Bt[:, 0:BS],
                        lhsT=xt16_sb[:, m, k, :],
                        rhs=wn16_sb[:, j, k, :],
                        start=(k == 0), stop=False,
                    )
                # block-diagonal h-projections: one fp8 DoubleRow each
                last = not has_bias
                nc.tensor.matmul(
                    A[:, :],
                    lhsT=ht_jp[:, m, 2 * half_i : 2 * half_i + 2, :],
                    rhs=whrz8_sb[:, :, j, :],
                    start=False, stop=last, perf_mode=DR,
                )
                nc.tensor.matmul(
                    Bt[:, BS : 2 * BS],
                    lhsT=ht_jp[:, m, 2 * half_i : 2 * half_i + 2, :],
                    rhs=whn8_sb[:, :, j, :],
                    start=False, stop=last, perf_mode=DR,
                )
                if has_bias:
                    nc.tensor.matmul(
                        A[:, :], lhsT=ones_sb[:, :], rhs=brz_sb[:, jrz],
                        start=False, stop=True,
                    )
                    nc.tensor.matmul(
                        Bt[:, 0:BS], lhsT=ones_sb[:, :], rhs=bxn_sb[:, jn],
                        start=False, stop=False,
                    )
                    nc.tensor.matmul(
                        Bt[:, BS : 2 * BS], lhsT=ones_sb[:, :], rhs=bhn_sb[:, jn],
                        start=False, stop=True,
                    )

                rz = epool.tile([P, 2 * BS], f16, tag="rz")
                nc.scalar.activation(rz[:], A[:, :], SIG, scale=1.0 / S)
                # epilogue is software-pipelined one (j,m) item behind the
                # matmuls: the in-order ACT queue would otherwise serialize
                # on the rz -> DVE t3/t4 -> tanh roundtrip (~2us, the same
                # as the PE's per-item rate) and accumulate a drain tail.
                if PIPELINE_EPILOGUE:
                    if pending is not None:
                        finish_epilogue(*pending)
                    pending = (j, m, rz, Bt, h_jp, ostage[m])
                else:
                    finish_epilogue(j, m, rz, Bt, h_jp, ostage[m])

        if pending is not None:
            finish_epilogue(*pending)

    nc.compile()
    _BUILD_CACHE[key] = nc
    return nc


def prep_inputs(x, h, W_ir, b_ir_lin, b_ir, W_h, b_hr, ncores=NCORES):
    """Host-side reshaping/casting -> per-core in_maps + has_bias flag."""
    import ml_dtypes

    f8 = ml_dtypes.float8_e4m3

    x = np.asarray(x, dtype=np.float32)
    h = np.asarray(h, dtype=np.float32)
    W_ir = np.asarray(W_ir, dtype=np.float32)
    W_h = np.asarray(W_h, dtype=np.float32)
    b_ir_lin = np.asarray(b_ir_lin, dtype=np.float32)
    b_ir = np.asarray(b_ir, dtype=np.float32)
    b_hr = np.asarray(b_hr, dtype=np.float32)

    bc = x.shape[0] // ncores
    K1 = IN // P
    K2 = BS // P
    MT = bc // P
    NJP = NB // 2

    # weights: gate-and-block reordered, x64 prescale, contraction-dim-major,
    # laid out so each per-block DMA slice is contiguous per partition line
    Wr = W_ir[0:H].reshape(NB, BS, IN)
    Wz = W_ir[H : 2 * H].reshape(NB, BS, IN)
    Wn_ = W_ir[2 * H :].reshape(NB, BS, IN)
    Wrz = np.concatenate([Wr, Wz], axis=1)  # [NB, 512, IN]
    # [P, NB, K1, 2BS]: [p, j, k, f] = S * Wrz[j, f, k*128+p]
    wrz8 = np.ascontiguousarray(
        (Wrz * S).reshape(NB, 2 * BS, K1, P).transpose(3, 0, 2, 1)
    ).astype(f8)
    wn16 = np.ascontiguousarray(
        (Wn_ * S).reshape(NB, BS, K1, P).transpose(3, 0, 2, 1)
    ).astype(np.float16)
    # [P, K2, NB, cols] from W_h slices (contraction = within-block h index)
    whrz8 = np.ascontiguousarray(
        (W_h[:, 0 : 2 * BS, :] * S).reshape(NB, 2 * BS, K2, P).transpose(3, 2, 0, 1)
    ).astype(f8)
    whn8 = np.ascontiguousarray(
        (W_h[:, 2 * BS :, :] * S).reshape(NB, BS, K2, P).transpose(3, 2, 0, 1)
    ).astype(f8)

    bx = b_ir_lin + b_ir
    bh = b_hr.reshape(NB, 3 * BS)
    brz = np.concatenate(
        [
            bx[0:H].reshape(NB, BS) + bh[:, 0:BS],
            bx[H : 2 * H].reshape(NB, BS) + bh[:, BS : 2 * BS],
        ],
        axis=1,
    ).reshape(1, NB * 2 * BS)
    bxn = bx[2 * H :].reshape(1, NB * BS).copy()
    bhn = bh[:, 2 * BS :].reshape(1, NB * BS).copy()
    has_bias = bool(np.any(brz) or np.any(bxn) or np.any(bhn))

    in_maps = []
    for c in range(ncores):
        csl = slice(c * bc, (c + 1) * bc)
        xc = x[csl]  # [bc, IN]
        hc = h[csl]  # [bc, H]
        # xt [P, MT, K1, 128]: [p, m, k, col] = x[m*128+col, k*128+p]
        xT = xc.T.reshape(K1, P, MT, P).transpose(1, 2, 0, 3)
        xt8 = np.ascontiguousarray(xT).astype(f8)
        xt16 = np.ascontiguousarray(xT).astype(np.float16)
        # ht8 [P, MT, NJP, 2K2, 128]: [p,m,jp,kk,c] = h[m*128+c, jp*512+kk*128+p]
        hT = hc.T.reshape(NJP, 2 * K2, P, MT, P).transpose(2, 3, 0, 1, 4)
        ht8 = np.ascontiguousarray(hT).astype(f8)
        m = {
            "xt8": xt8,
            "xt16": xt16,
            "ht8": ht8,
            "h16": np.ascontiguousarray(hc).astype(np.float16),
            "wrz8": wrz8,
            "wn16": wn16,
            "whrz8": whrz8,
            "whn8": whn8,
        }
        if has_bias:
            m["brz"] = (brz * S).astype(np.float32)
            m["bxn"] = (bxn * S).astype(np.float32)
            m["bhn"] = (bhn * S).astype(np.float32)
        in_maps.append(m)
    return in_maps, has_bias, bc


def kernel(x, h, W_ir, b_ir_lin, b_ir, W_h, b_hr):
    from concourse.bass_utils import run_bass_kernel_spmd

    in_maps, has_bias, bc = prep_inputs(x, h, W_ir, b_ir_lin, b_ir, W_h, b_hr)
    nc = build_nc(bc=bc, has_bias=has_bias)
    try:
        res = run_bass_kernel_spmd(nc, in_maps, list(range(NCORES)))
    except Exception:
        # transient NRT device errors have been observed once in ~10 runs;
        # a single retry reuses the compiled NEFF
        res = run_bass_kernel_spmd(nc, in_maps, list(range(NCORES)))
    return np.concatenate(
        [res.results[c]["out"].astype(np.float32) for c in range(NCORES)], axis=0
    )


# revision 40
# speedup vs baseline: 1.0235x; 1.0235x over previous
"""Block-diagonal GRU cell for Trainium2, data-parallel over 8 NeuronCores.

Math (per batch row b, block j of 8, block size 256):
    wx  = x @ W_ir.T + b_ir_lin + b_ir          # [B, 6144], gates r|z|n global-chunked
    wh  = hb_j @ W_h[j].T + b_hr_j              # per block, local r|z|n chunks of 256
    r   = sigmoid(wxr + whr)
    z   = sigmoid(wxz + whz)
    n   = tanh(wxn + r * whn)
    h'  = (1-z)*hb + z*n

Device strategy (per core, batch-sharded 1024 rows):
  - Mixed fp8/fp16 matmuls. The r/z gates (x- and h-projections) and the
    n-gate h-projection run in fp8e4m3 with MatmulPerfMode.DoubleRow
    (2 contraction rows/cycle = 2x bf16 rate); the n-gate x-projection
    stays fp16 (it dominates output error: all-fp8 sims at rel 2.1e-2 vs
    the 2e-2 budget, this mix sims at 1.24e-2). PE work: 311296 cycles
    ~= 130us vs 205us all-fp16.
  - fp8 weights are pre-scaled by S=64 on host (W~N(0,0.02) sits at the
    e4m3 subnormal boundary; x64 moves it to N(0,1.28)). The 1/S folds
    into the activation `scale` operand. The fp16 n-gate weights get the
    same x64 so PSUM bank B is uniformly scaled.
  - PSUM bank A [128,512] accumulates 64*(wxr+whr | wxz+whz) in one
    accumulation group (4 DoubleRow x-chunks + 1 DoubleRow h-chunk).
    Bank B holds 64*[wxn | whn] (whn separate: r multiplies only whn).
  - Epilogue is fp16 (2-byte dtypes unlock DVE 2x perf modes) and uses
    the real Tanh table (same ACT table set as Sigmoid on trn2):
        rz = sigmoid(A/S)            ACT
        t3 = r * B.whn               DVE   (= S*(r*whn))
        t4 = B.wxn + t3              DVE
        n  = tanh(t4/S)              ACT
        d  = n - hb                  DVE
        t5 = z * d                   DVE
        oj = t5 + hb                 GPSIMD
    Output is stored fp16 and upcast on host (rounding ~2e-4, noise
    floor here is ~1.2e-2 from fp8).
  - Host layouts make every DMA contiguous >=1KB per partition line:
    xt8/xt16 [P, MT, K1, 128] (per-m-tile slices), ht8 [P, MT, JP, 4, 128]
    (per (m, block-pair) slices), weights [P, NB, K1, cols] (per-block
    slices), whrz/whn loaded whole.
  - Loop nest is j-outer / m-inner with j-column-major weight DMAs and
    per-(m,block-pair) stream tiles; next-block weights prefetch at the
    top of the previous block's m-loop, next-pair streams right after
    the slots they need release.
  - The epilogue is software-pipelined one (j,m) item behind the
    matmuls: the in-order ACT queue otherwise serializes on the
    rz -> DVE t3/t4 -> tanh roundtrip (~2us/item, the same as the PE's
    rate) and drains a ~10us tail after the last matmul.
  - Loads ride the SP HWDGE ring; stores are batched per (m, pair)
    [128,512] fp16 on the ACT HWDGE ring. Measured pitfall: gpsimd
    dma_start is software-DGE (~1us of Pool-sequencer descriptor
    generation per store) — keep stores on a HWDGE ring (SP/ACT).

  - j0's loads are k-split and ordered by the in-order PE queue's need
    times (the PE stalls on ANY missing operand of the next queued
    matmul). Keep the many small per-m/per-(m,jp) triggers: one
    consolidated big DMA serializes on a single DMA engine and
    measured ~4us SLOWER than 16 small triggers despite the ~600ns
    per-trigger SP-queue cost.

Measured on 8 trn2 cores: 163650 ns best, rel err 1.2446e-2
(sim-predicted 1.2446e-2; fixed seed makes this deterministic). The
same NEFF lands bimodally at ~164us or ~190us depending on chip clock
state at launch (all engines uniformly ~18% slower in the slow state;
not caused by the program). bf16 baseline was 241842 ns.
"""

import sys

if "/opt/trn_rl_repo" not in sys.path:
    sys.path.insert(0, "/opt/trn_rl_repo")

import numpy as np

B, IN, H, NB = 8192, 1024, 2048, 8
BS = H // NB  # 256
NCORES = 8
BC = B // NCORES  # 1024 rows per core
P = 128
S = 64.0  # fp8 weight prescale
PIPELINE_EPILOGUE = True

_BUILD_CACHE = {}


def build_nc(bc=BC, has_bias=False):
    """Build the Bass program for one core (SPMD: same program on all 8)."""
    key = (bc, has_bias)
    if key in _BUILD_CACHE:
        return _BUILD_CACHE[key]

    from contextlib import ExitStack

    import concourse.bacc as bacc
    import concourse.mybir as mybir
    import concourse.tile as tile

    f8 = mybir.dt.float8e4
    f16 = mybir.dt.float16
    f32 = mybir.dt.float32
    SIG = mybir.ActivationFunctionType.Sigmoid
    TANH = mybir.ActivationFunctionType.Tanh
    DR = mybir.MatmulPerfMode.DoubleRow

    K1 = IN // P  # 8 contraction chunks for the x projection
    K2 = BS // P  # 2 contraction chunks per block for the h projection
    MT = bc // P  # m-tiles (128 batch rows each)
    NJP = NB // 2  # block pairs

    nc = bacc.Bacc(target_bir_lowering=False)

    xt8_d = nc.dram_tensor("xt8", [P, MT, K1, P], f8, kind="ExternalInput").ap()
    xt16_d = nc.dram_tensor("xt16", [P, MT, K1, P], f16, kind="ExternalInput").ap()
    ht8_d = nc.dram_tensor("ht8", [P, MT, NJP, 2 * K2, P], f8, kind="ExternalInput").ap()
    h16_d = nc.dram_tensor("h16", [bc, H], f16, kind="ExternalInput").ap()
    wrz8_d = nc.dram_tensor("wrz8", [P, NB, K1, 2 * BS], f8, kind="ExternalInput").ap()
    wn16_d = nc.dram_tensor("wn16", [P, NB, K1, BS], f16, kind="ExternalInput").ap()
    whrz8_d = nc.dram_tensor("whrz8", [P, K2, NB, 2 * BS], f8, kind="ExternalInput").ap()
    whn8_d = nc.dram_tensor("whn8", [P, K2, NB, BS], f8, kind="ExternalInput").ap()
    if has_bias:
        brz_d = nc.dram_tensor("brz", [1, NB * 2 * BS], f32, kind="ExternalInput").ap()
        bxn_d = nc.dram_tensor("bxn", [1, NB * BS], f32, kind="ExternalInput").ap()
        bhn_d = nc.dram_tensor("bhn", [1, NB * BS], f32, kind="ExternalInput").ap()
    out = nc.dram_tensor("out", [bc, H], f16, kind="ExternalOutput").ap()

    with tile.TileContext(nc) as tc, ExitStack() as ctx:
        wpool = ctx.enter_context(tc.tile_pool(name="wres", bufs=1))
        spool = ctx.enter_context(tc.tile_pool(name="stream", bufs=MT + MT // 2))
        psA = ctx.enter_context(tc.tile_pool(name="psA", bufs=4, space="PSUM"))
        psB = ctx.enter_context(tc.tile_pool(name="psB", bufs=4, space="PSUM"))
        epool = ctx.enter_context(tc.tile_pool(name="epi", bufs=4))
        opool = ctx.enter_context(tc.tile_pool(name="ostage", bufs=MT + 2))

        # ---- resident tiles ----
        xt8_sb = wpool.tile([P, MT, K1, P], f8, tag="xt8_sb")
        xt16_sb = wpool.tile([P, MT, K1, P], f16, tag="xt16_sb")
        wrz8_sb = wpool.tile([P, NB, K1, 2 * BS], f8, tag="wrz8_sb")
        wn16_sb = wpool.tile([P, NB, K1, BS], f16, tag="wn16_sb")
        whrz8_sb = wpool.tile([P, K2, NB, 2 * BS], f8, tag="whrz8_sb")
        whn8_sb = wpool.tile([P, K2, NB, BS], f8, tag="whn8_sb")

        def load_w_cols(j, ksplit=False):
            if ksplit:
                # j0 head sequence, ordered by the in-order PE queue's need
                # times: wrz8 k-chunks (A x-proj), then B-bank operands
                # (xt16/wn16/whn8 — the PE reaches B(m0) ~0.5us in), then
                # whrz8 (A-bank stop; 4-deep psA buys ~4us of slack)
                for k in range(K1 // 2):
                    nc.sync.dma_start(
                        wrz8_sb[:, j, 2 * k : 2 * k + 2], wrz8_d[:, j, 2 * k : 2 * k + 2]
                    )
                nc.sync.dma_start(xt16_sb[:, 0], xt16_d[:, 0])
                nc.sync.dma_start(
                    wn16_sb[:, j, 0 : K1 // 2], wn16_d[:, j, 0 : K1 // 2]
                )
                nc.sync.dma_start(whn8_sb[:, :, j], whn8_d[:, :, j])
                nc.sync.dma_start(
                    wn16_sb[:, j, K1 // 2 :], wn16_d[:, j, K1 // 2 :]
                )
                nc.sync.dma_start(whrz8_sb[:, :, j], whrz8_d[:, :, j])
            else:
                nc.sync.dma_start(wrz8_sb[:, j], wrz8_d[:, j])
                nc.sync.dma_start(wn16_sb[:, j], wn16_d[:, j])
                nc.sync.dma_start(whrz8_sb[:, :, j], whrz8_d[:, :, j])
                nc.sync.dma_start(whn8_sb[:, :, j], whn8_d[:, :, j])

        def load_mp_streams(m, jp):
            # per-(m, pair) small loads: many small triggers beat few big
            # DMAs here — one trigger's descriptors stay on one DMA engine,
            # so big consolidated transfers serialize (measured +4us)
            ht_mp = spool.tile([P, 2 * K2, P], f8, tag="ht_mp")
            nc.sync.dma_start(ht_mp[:], ht8_d[:, m, jp])
            h_mp = spool.tile([P, 2 * BS], f16, tag="h_mp")
            msl = slice(m * P, (m + 1) * P)
            psl = slice(2 * jp * BS, (2 * jp + 2) * BS)
            nc.sync.dma_start(h_mp[:], h16_d[msl, psl])
            return ht_mp, h_mp

        # prewarm the ACT table set (sigmoid_and_others contains Tanh too)
        ws = wpool.tile([P, 1], f32, tag="ws")
        nc.vector.memset(ws[:], 0.0)
        nc.scalar.activation(ws[:], ws[:], SIG)

        # head: feed the PE from ~2.5us on. The first DoubleRow needs only
        # xt8[0] + wrz8[0,k0] (256KB); h-weights load per-block columns so
        # the critical first-pipeline bytes stay small (~2.2MB for the
        # full (j0,m0) A+B pipeline instead of ~3.6MB).
        nc.sync.dma_start(xt8_sb[:, 0], xt8_d[:, 0])
        streams = {}
        streams[(0, 0)] = load_mp_streams(0, 0)
        load_w_cols(0, ksplit=True)
        for m in range(1, MT):
            nc.sync.dma_start(xt8_sb[:, m], xt8_d[:, m])
            nc.sync.dma_start(xt16_sb[:, m], xt16_d[:, m])
            streams[(m, 0)] = load_mp_streams(m, 0)
        if has_bias:
            ones_sb = wpool.tile([1, P], f32, tag="ones_sb")
            nc.vector.memset(ones_sb[:], 1.0)
            brz_sb = wpool.tile([1, NB * 2 * BS], f32, tag="brz_sb")
            bxn_sb = wpool.tile([1, NB * BS], f32, tag="bxn_sb")
            bhn_sb = wpool.tile([1, NB * BS], f32, tag="bhn_sb")
            nc.sync.dma_start(brz_sb[:], brz_d[:])
            nc.sync.dma_start(bxn_sb[:], bxn_d[:])
            nc.sync.dma_start(bhn_sb[:], bhn_d[:])

        ostage = {}
        pending = None

        def finish_epilogue(j, m, rz, Bt, h_mp, oj2):
            jp, half_i = divmod(j, 2)
            half = slice(half_i * BS, (half_i + 1) * BS)
            msl = slice(m * P, (m + 1) * P)
            t3 = epool.tile([P, BS], f16, tag="t3")
            nc.vector.tensor_mul(t3[:], rz[:, 0:BS], Bt[:, BS : 2 * BS])
            t4 = epool.tile([P, BS], f16, tag="t4")
            nc.vector.tensor_add(t4[:], Bt[:, 0:BS], t3[:])
            tn = epool.tile([P, BS], f16, tag="tn")
            nc.scalar.activation(tn[:], t4[:], TANH, scale=1.0 / S)
            d = epool.tile([P, BS], f16, tag="d")
            nc.vector.tensor_sub(d[:], tn[:], h_mp[:, half])
            t5 = epool.tile([P, BS], f16, tag="t5")
            nc.vector.tensor_mul(t5[:], rz[:, BS : 2 * BS], d[:])
            nc.gpsimd.tensor_add(oj2[:, half], t5[:], h_mp[:, half])
            if half_i == 1:
                # one batched [128, 512] fp16 store per (m, block-pair) on
                # the ACT HWDGE ring. (gpsimd DMAs are software-DGE: ~1us
                # of Pool-sequencer descriptor generation each -- avoid.)
                psl = slice(2 * jp * BS, (2 * jp + 2) * BS)
                nc.scalar.dma_start(out[msl, psl], oj2[:, :])
                del ostage[m]
                streams.pop((m, jp, "cur"))
                # this m's pair tiles just released: prefetch its
                # next-pair streams now
                if jp + 1 < NJP:
                    streams[(m, jp + 1)] = load_mp_streams(m, jp + 1)

        for j in range(NB):
            jp, half_i = divmod(j, 2)
            jrz = slice(j * 2 * BS, (j + 1) * 2 * BS)
            jn = slice(j * BS, (j + 1) * BS)
            half = slice(half_i * BS, (half_i + 1) * BS)
            # prefetch the next block's weights ahead of the next pair's
            # stream loads on the SP queue
            if j + 1 < NB:
                load_w_cols(j + 1)
            for m in range(MT):
                msl = slice(m * P, (m + 1) * P)
                if half_i == 0:
                    streams[(m, jp, "cur")] = streams.pop((m, jp))
                    ostage[m] = opool.tile(
                        [P, 2 * BS], f16, tag="oj2", name=f"oj2_{m}"
                    )
                ht_mp, h_mp = streams[(m, jp, "cur")]
                A = psA.tile([P, 2 * BS], f32, tag="A")
                Bt = psB.tile([P, 2 * BS], f32, tag="B")
                # rz x-projection: fp8 DoubleRow over k-chunk pairs; B's
                # group start marks the whole bank pending-zero so the
                # h-side MMs overwrite-then-accumulate correctly.
                for k in range(K1 // 2):
                    nc.tensor.matmul(
                        A[:, :],
                        lhsT=xt8_sb[:, m, 2 * k : 2 * k + 2, :],
                        rhs=wrz8_sb[:, j, 2 * k : 2 * k + 2, :],
                        start=(k == 0), stop=False, perf_mode=DR,
                    )
                # n x-projection: fp16 (error-critical path)
                for k in range(K1):
                    nc.tensor.matmul(
                        Bt[:, 0:BS],
                        lhsT=xt16_sb[:, m, k, :],
                        rhs=wn16_sb[:, j, k, :],
                        start=(k == 0), stop=False,
                    )
                # block-diagonal h-projections: one fp8 DoubleRow each
                last = not has_bias
                nc.tensor.matmul(
                    A[:, :],
                    lhsT=ht_mp[:, 2 * half_i : 2 * half_i + 2, :],
                    rhs=whrz8_sb[:, :, j, :],
                    start=False, stop=last, perf_mode=DR,
                )
                nc.tensor.matmul(
                    Bt[:, BS : 2 * BS],
                    lhsT=ht_mp[:, 2 * half_i : 2 * half_i + 2, :],
                    rhs=whn8_sb[:, :, j, :],
                    start=False, stop=last, perf_mode=DR,
                )
                if has_bias:
                    nc.tensor.matmul(
                        A[:, :], lhsT=ones_sb[:, :], rhs=brz_sb[:, jrz],
                        start=False, stop=True,
                    )
                    nc.tensor.matmul(
                        Bt[:, 0:BS], lhsT=ones_sb[:, :], rhs=bxn_sb[:, jn],
                        start=False, stop=False,
                    )
                    nc.tensor.matmul(
                        Bt[:, BS : 2 * BS], lhsT=ones_sb[:, :], rhs=bhn_sb[:, jn],
                        start=False, stop=True,
                    )

                rz = epool.tile([P, 2 * BS], f16, tag="rz")
                nc.scalar.activation(rz[:], A[:, :], SIG, scale=1.0 / S)
                # epilogue is software-pipelined one (j,m) item behind the
                # matmuls: the in-order ACT queue would otherwise serialize
                # on the rz -> DVE t3/t4 -> tanh roundtrip (~2us, the same
                # as the PE's per-item rate) and accumulate a drain tail.
                if PIPELINE_EPILOGUE:
                    if pending is not None:
                        finish_epilogue(*pending)
                    pending = (j, m, rz, Bt, h_mp, ostage[m])
                else:
                    finish_epilogue(j, m, rz, Bt, h_mp, ostage[m])

        if pending is not None:
            finish_epilogue(*pending)

    nc.compile()
    _BUILD_CACHE[key] = nc
    return nc


def prep_inputs(x, h, W_ir, b_ir_lin, b_ir, W_h, b_hr, ncores=NCORES):
    """Host-side reshaping/casting -> per-core in_maps + has_bias flag."""
    import ml_dtypes

    f8 = ml_dtypes.float8_e4m3

    x = np.asarray(x, dtype=np.float32)
    h = np.asarray(h, dtype=np.float32)
    W_ir = np.asarray(W_ir, dtype=np.float32)
    W_h = np.asarray(W_h, dtype=np.float32)
    b_ir_lin = np.asarray(b_ir_lin, dtype=np.float32)
    b_ir = np.asarray(b_ir, dtype=np.float32)
    b_hr = np.asarray(b_hr, dtype=np.float32)

    bc = x.shape[0] // ncores
    K1 = IN // P
    K2 = BS // P
    MT = bc // P
    NJP = NB // 2

    # weights: gate-and-block reordered, x64 prescale, contraction-dim-major,
    # laid out so each per-block DMA slice is contiguous per partition line
    Wr = W_ir[0:H].reshape(NB, BS, IN)
    Wz = W_ir[H : 2 * H].reshape(NB, BS, IN)
    Wn_ = W_ir[2 * H :].reshape(NB, BS, IN)
    Wrz = np.concatenate([Wr, Wz], axis=1)  # [NB, 512, IN]
    # [P, NB, K1, 2BS]: [p, j, k, f] = S * Wrz[j, f, k*128+p]
    wrz8 = np.ascontiguousarray(
        (Wrz * S).reshape(NB, 2 * BS, K1, P).transpose(3, 0, 2, 1)
    ).astype(f8)
    wn16 = np.ascontiguousarray(
        (Wn_ * S).reshape(NB, BS, K1, P).transpose(3, 0, 2, 1)
    ).astype(np.float16)
    # [P, K2, NB, cols] from W_h slices (contraction = within-block h index)
    whrz8 = np.ascontiguousarray(
        (W_h[:, 0 : 2 * BS, :] * S).reshape(NB, 2 * BS, K2, P).transpose(3, 2, 0, 1)
    ).astype(f8)
    whn8 = np.ascontiguousarray(
        (W_h[:, 2 * BS :, :] * S).reshape(NB, BS, K2, P).transpose(3, 2, 0, 1)
    ).astype(f8)

    bx = b_ir_lin + b_ir
    bh = b_hr.reshape(NB, 3 * BS)
    brz = np.concatenate(
        [
            bx[0:H].reshape(NB, BS) + bh[:, 0:BS],
            bx[H : 2 * H].reshape(NB, BS) + bh[:, BS : 2 * BS],
        ],
        axis=1,
    ).reshape(1, NB * 2 * BS)
    bxn = bx[2 * H :].reshape(1, NB * BS).copy()
    bhn = bh[:, 2 * BS :].reshape(1, NB * BS).copy()
    has_bias = bool(np.any(brz) or np.any(bxn) or np.any(bhn))

    in_maps = []
    for c in range(ncores):
        csl = slice(c * bc, (c + 1) * bc)
        xc = x[csl]  # [bc, IN]
        hc = h[csl]  # [bc, H]
        # xt [P, MT, K1, 128]: [p, m, k, col] = x[m*128+col, k*128+p]
        xT = xc.T.reshape(K1, P, MT, P).transpose(1, 2, 0, 3)
        xt8 = np.ascontiguousarray(xT).astype(f8)
        xt16 = np.ascontiguousarray(xT).astype(np.float16)
        # ht8 [P, MT, NJP, 2K2, 128]: [p,m,jp,kk,c] = h[m*128+c, jp*512+kk*128+p]
        hT = hc.T.reshape(NJP, 2 * K2, P, MT, P).transpose(2, 3, 0, 1, 4)
        ht8 = np.ascontiguousarray(hT).astype(f8)
        m = {
            "xt8": xt8,
            "xt16": xt16,
            "ht8": ht8,
            "h16": np.ascontiguousarray(hc).astype(np.float16),
            "wrz8": wrz8,
            "wn16": wn16,
            "whrz8": whrz8,
            "whn8": whn8,
        }
        if has_bias:
            m["brz"] = (brz * S).astype(np.float32)
            m["bxn"] = (bxn * S).astype(np.float32)
            m["bhn"] = (bhn * S).astype(np.float32)
        in_maps.append(m)
    return in_maps, has_bias, bc


def kernel(x, h, W_ir, b_ir_lin, b_ir, W_h, b_hr):
    from concourse.bass_utils import run_bass_kernel_spmd

    in_maps, has_bias, bc = prep_inputs(x, h, W_ir, b_ir_lin, b_ir, W_h, b_hr)
    nc = build_nc(bc=bc, has_bias=has_bias)
    try:
        res = run_bass_kernel_spmd(nc, in_maps, list(range(NCORES)))
    except Exception:
        # transient NRT device errors have been observed once in ~10 runs;
        # a single retry reuses the compiled NEFF
        res = run_bass_kernel_spmd(nc, in_maps, list(range(NCORES)))
    return np.concatenate(
        [res.results[c]["out"].astype(np.float32) for c in range(NCORES)], axis=0
    )
